# revision 6
# baseline (speedup 1.0000x reference)
"""Quantized 3x3 conv via 1D Winograd F(2,3) on 8 TRN2 NeuronCores.

Reference semantics:
    qx = clip(round(x * (127/3)), -127, 127)          # int values in f32
    qw = clip(round(w * (127/0.05)), -127, 127)
    out = conv2d(qx, qw, stride 1, pad 1) * (3*0.05/127^2) + bias

Strategy: data-parallel over batch (4 images/core). The W-dimension conv
uses Winograd F(2,3): 4 transform points per 2 outputs cut PE columns to
2/3 of direct (9 taps -> 12 passes of half-width tiles). Transform math is
fp16: transformed activations are integers <= 254, exact in fp16; RESCALE
is folded into the transformed weights. Host-validated rel err ~4e-4
(tolerance 2e-2).

Per core:
  - x ships as bf16 (halves input DMA), quantized via ACT (scale + magic
    round) + DVE/GpSimd clamps into zero-padded [58,61] fp16 images.
  - in-transform: 4 tensor_tensor fp16 ops per image build
    xt[p=4, y=58, t=28] from stride-2 views of xq.
  - conv: per (img, 14-row block, cout-chunk): 12 matmuls (4 points x 3
    ky) of N=392 accumulate m-points into f32 PSUM (4 tags x 2 bufs = 8
    banks).
  - inverse transform: ACT evacuates m1(+bias)/m2 to fp16 SBUF (GpSimd
    cannot touch PSUM), GpSimd forms X=sA+c2 / Y=sA-c2, DVE finishes
    o0 = m0+X, o1 = Y-m3 (one PSUM operand per op). Outputs interleave
    fp16 into [rows,56] tiles, DMA out on the ACT ring; host casts f32.
"""

import numpy as np
import ml_dtypes

import concourse.mybir as mybir
import concourse.tile as tile
from concourse import bacc
from concourse.bass_utils import run_bass_kernel_spmd

B, CIN, COUT, H, W, KS = 32, 128, 256, 56, 56, 3
NCORES = 8
BPC = B // NCORES
NPIX = H * W
NTAP = KS * KS
NCHUNK = COUT // 128
QL = 127.0
SX = QL / 3.0
SW = QL / 0.05
RESCALE = (3.0 * 0.05) / (QL * QL)
MAGIC = 1.5 * 2.0**23

NP_ = 4                     # winograd points
TW = 28                     # tiles along W (56/2)
HP = H + 2                  # padded rows (58)
WP = 61                     # padded cols (58 used + slack for views)
NBLK = 4                    # 14-row blocks per image
BR = H // NBLK              # 14
NCOL = BR * TW              # 392

F32 = mybir.dt.float32
F16 = mybir.dt.float16
BF16 = mybir.dt.bfloat16
AL = mybir.AluOpType

_NC = None
X_NP_DTYPE = ml_dtypes.bfloat16


def _rs(ap, nr):
    return ap.rearrange("p (y t) -> p y t", y=nr)


def _build(reps: int = 1):
    nc = bacc.Bacc("TRN2", target_bir_lowering=False, num_devices=NCORES)

    x_t = nc.dram_tensor("x", [BPC, CIN, NPIX], BF16, kind="ExternalInput")
    w_t = nc.dram_tensor("weight", [NTAP, CIN, COUT], F32, kind="ExternalInput")
    b_t = nc.dram_tensor("bias", [NCHUNK, 128, 1], F32, kind="ExternalInput")
    o_t = nc.dram_tensor("out", [BPC, NCHUNK, 128, 2, NPIX // 2], F16,
                         kind="ExternalOutput")

    with tile.TileContext(nc) as tc:
        with (
            tc.tile_pool(name="consts", bufs=1) as consts,
            tc.tile_pool(name="xq", bufs=1) as xqp,
            tc.tile_pool(name="xt", bufs=1) as xtp,
            tc.tile_pool(name="xstage", bufs=2) as xsp,
            tc.tile_pool(name="tmp", bufs=2) as tmpp,
            tc.tile_pool(name="sdef", bufs=3) as sdefp,
            tc.tile_pool(name="outp", bufs=4) as outp,
            tc.tile_pool(name="psum", bufs=2, space="PSUM") as psp,
        ):
            # ---- padded quantized images [58, 61] fp16; borders zero once
            xq = []
            for b in range(BPC):
                t = xqp.tile([128, HP, WP], F16, tag=f"xq{b}")
                nc.gpsimd.memset(t[:, 0, :], 0.0)
                nc.gpsimd.memset(t[:, HP - 1, :], 0.0)
                nc.gpsimd.memset(t[:, 1:HP - 1, 0], 0.0)
                nc.gpsimd.memset(t[:, 1:HP - 1, 57:WP], 0.0)
                xq.append(t)
            xt = [xtp.tile([128, NP_, HP, TW], F16, tag=f"xt{b}",
                           name=f"xt{b}")
                  for b in range(BPC)]

            # ---- weights: DMA [ci, tap, co] f32, quantize to exact ints,
            # F(2,3) G-transform (x RESCALE) into wt [ci, p, ky, co] fp16 --
            wraw = consts.tile([128, NTAP, COUT], F32, tag="wraw")
            nc.scalar.dma_start(
                out=wraw[:], in_=w_t[:].rearrange("t p c -> p t c"))
            qw = consts.tile([128, NTAP, COUT], F32, tag="qw")
            nc.scalar.activation(
                qw[:], wraw[:], mybir.ActivationFunctionType.Copy,
                bias=MAGIC, scale=SW)
            nc.gpsimd.tensor_scalar(qw[:], qw[:], MAGIC, -QL,
                                    AL.subtract, AL.max)
            nc.gpsimd.tensor_scalar_min(qw[:], qw[:], QL)

            wt = consts.tile([128, NP_, KS, COUT], F16, tag="wt")
            wtmp = consts.tile([128, 2, COUT], F32, tag="wtmp")
            RS = RESCALE
            for ky in range(KS):
                w0, w1, w2 = (qw[:, ky * 3 + kx, :] for kx in range(3))
                u = wtmp[:, 0, :]
                v = wtmp[:, 1, :]
                # p0 = RS w0 ; p3 = RS w2
                nc.vector.tensor_scalar_mul(wt[:, 0, ky, :], w0, RS)
                nc.vector.tensor_scalar_mul(wt[:, 3, ky, :], w2, RS)
                # p1 = RS/2 (w0+w1+w2); p2 = RS/2 (w0-w1+w2)
                nc.vector.tensor_add(u, w0, w2)
                nc.vector.tensor_add(v, u, w1)
                nc.vector.tensor_scalar_mul(wt[:, 1, ky, :], v, RS / 2)
                nc.vector.tensor_sub(v, u, w1)
                nc.vector.tensor_scalar_mul(wt[:, 2, ky, :], v, RS / 2)

            bias_sb = []
            for c in range(NCHUNK):
                bs = consts.tile([128, 1], F32, tag=f"bias{c}")
                nc.scalar.dma_start(out=bs[:], in_=b_t[c])
                bias_sb.append(bs)

            # ---- PE warmup (clock gate) ----
            warm = consts.tile([128, NCOL], BF16, tag="warm")
            nc.gpsimd.memset(warm[:], 1.0)
            wpt = psp.tile([128, NCOL], F32, tag="mp0", name="warm_pt")
            for i in range(14):
                nc.tensor.matmul(wpt[:], warm[:, 0:128], warm[:],
                                 start=True, stop=True)

            def body(_iv=None):
                # ---- input DMA + quantization, 2 halves per image ----
                for b in range(BPC):
                    for hi, (r0, nr) in enumerate(((0, 28), (28, 28))):
                        xs = xsp.tile([128, 28 * W], BF16, tag=f"xs{hi}",
                                      name=f"xs{b}_{hi}")
                        nc.sync.dma_start(
                            out=xs[:], in_=x_t[b, :, r0 * W:(r0 + nr) * W])
                        t1 = tmpp.tile([128, 28 * W], F32, tag=f"t1_{hi}",
                                       name=f"t1_{b}_{hi}")
                        nc.scalar.activation(
                            t1[:], xs[:], mybir.ActivationFunctionType.Copy,
                            bias=MAGIC, scale=SX)
                        t2 = tmpp.tile([128, 28 * W], F16, tag=f"t2_{hi}",
                                       name=f"t2_{b}_{hi}")
                        nc.scalar.activation(
                            t2[:], t1[:], mybir.ActivationFunctionType.Copy,
                            bias=-MAGIC, scale=1.0)
                        meng = nc.vector if b < 2 else nc.gpsimd
                        meng.tensor_scalar(
                            xq[b][:, 1 + r0:1 + r0 + nr, 1:1 + W],
                            t2[:].rearrange("p (h w) -> p h w", h=nr),
                            -QL, QL, AL.max, AL.min)

                # ---- in-transform: 4 fp16 tensor ops per image -> xt ----
                for b in range(BPC):
                    eng = nc.vector if b < 3 else nc.gpsimd

                    def dk(k, _b=b):
                        v = xq[_b][:, :, k:k + 56]
                        return v.rearrange("p y (t f) -> p y t f", f=2)[
                            :, :, :, 0]

                    X = xt[b]
                    d0, d1, d2, d3 = (dk(k) for k in range(4))
                    eng.tensor_sub(X[:, 0], d0, d2)
                    eng.tensor_add(X[:, 1], d1, d2)
                    eng.tensor_sub(X[:, 2], d2, d1)
                    eng.tensor_sub(X[:, 3], d1, d3)

                # ---- conv + inverse transform per (img, block, chunk) ----
                for b in range(BPC):
                    for bi in range(NBLK):
                        r0 = bi * BR
                        for c in range(NCHUNK):
                            mp = [psp.tile([128, NCOL], F32, tag=f"mp{p}",
                                           name=f"mp{b}_{bi}_{c}_{p}")
                                  for p in range(NP_)]
                            for p in range(NP_):
                                for ky in range(KS):
                                    nc.tensor.matmul(
                                        mp[p][:],
                                        wt[:, p, ky, c * 128:(c + 1) * 128],
                                        xt[b][:, p, r0 + ky:r0 + ky + BR, :],
                                        start=(ky == 0), stop=(ky == 2),
                                    )
                            sd = sdefp.tile([128, 4, NCOL], F16, tag="sd",
                                            name=f"sd{b}_{bi}_{c}")
                            sA, c2, Xs, Ys = (sd[:, i] for i in range(4))
                            # ACT evacuates m1 (+bias) and m2 to fp16 SBUF
                            nc.scalar.activation(
                                sA, mp[1][:],
                                mybir.ActivationFunctionType.Identity,
                                bias=bias_sb[c][:], scale=1.0)
                            nc.scalar.activation(
                                c2, mp[2][:],
                                mybir.ActivationFunctionType.Copy,
                                bias=0.0, scale=1.0)
                            # X = sA+c2, Y = sA-c2
                            xy_eng = nc.gpsimd if b == 3 else nc.vector
                            xy_eng.tensor_add(Xs, sA, c2)
                            xy_eng.tensor_sub(Ys, sA, c2)
                            # DVE: o0 = m0 + X ; o1 = Y - m3 (interleaved
                            # SBUF writes; one contiguous DMA out)
                            ot = outp.tile([128, 2, NCOL], F16, tag="ot",
                                           name=f"ot{b}_{bi}_{c}")
                            nc.vector.tensor_add(ot[:, 0], mp[0][:], Xs)
                            nc.vector.tensor_sub(ot[:, 1], Ys, mp[3][:])
                            nc.scalar.dma_start(
                                out=o_t[b, c, :, :,
                                        r0 * TW:(r0 + BR) * TW],
                                in_=ot[:],
                            )

            if reps == 1:
                body()
            else:
                with tc.For_i(0, reps, 1):
                    body()
    nc.compile()
    return nc


def _get_nc():
    global _NC
    if _NC is None:
        _NC = _build()
    return _NC


def kernel(x: np.ndarray, weight: np.ndarray, bias: np.ndarray) -> np.ndarray:
    x = np.ascontiguousarray(
        np.asarray(x, dtype=np.float32).reshape(B, CIN, NPIX)
    ).astype(ml_dtypes.bfloat16)
    w_l = np.ascontiguousarray(
        np.asarray(weight, dtype=np.float32).transpose(2, 3, 1, 0)
    ).reshape(NTAP, CIN, COUT)
    b_l = np.ascontiguousarray(
        np.asarray(bias, dtype=np.float32)).reshape(NCHUNK, 128, 1)

    nc = _get_nc()
    in_maps = [
        {
            "x": np.ascontiguousarray(x[i * BPC:(i + 1) * BPC]),
            "weight": w_l,
            "bias": b_l,
        }
        for i in range(NCORES)
    ]
    res = run_bass_kernel_spmd(nc, in_maps, core_ids=list(range(NCORES)))
    planes = np.concatenate([r["out"] for r in res.results], axis=0)
    # [B, NCHUNK, 128, 2, H*W/2] -> interleave parity j into x = 2t+j
    planes = planes.reshape(B, COUT, 2, H, W // 2)
    out = np.ascontiguousarray(
        planes.transpose(0, 1, 3, 4, 2)).reshape(B, COUT, H, W)
    return out.astype(np.float32)
